# revision 7
# baseline (speedup 1.0000x reference)
"""Block-tensorized linear (TT-factored block linear) on 8 Trainium2 NeuronCores.

Problem (hardcoded shapes):
    x:    (4, 2048, 4096) fp32   -> 8192 tokens, 4096 features
    U:    (4, 4, 1024, 256) fp32 (rows, cols, block, rank)
    V:    (4, 4, 256, 1024) fp32 (rows, cols, rank, block)
    bias: (4, 1024) fp32
    y[t, o*1024+m] = sum_c sum_r (sum_v x[t, c*1024+v] V[o,c,r,v]) U[o,c,m,r] + bias[o,m]

Sharding: 2-way tensor parallel over output row-block pairs (cores 0-3 take
o in {0,1}, cores 4-7 take o in {2,3}) x 4-way data parallel over tokens
(2048 tokens per core). Each core keeps its transposed U/V resident in SBUF
and streams its token shard.

Both matmul stages run in fp16 (full PE rate, same as bf16/f32r, but half the
HBM traffic and SBUF footprint of f32r; FWL applies to the 16-bit weight
loads so they hide under the matmuls). PSUM accumulation is fp32, so the only
precision loss is input/intermediate quantization to fp16 (~2e-4 end-to-end
rel err). Tokens are the moving dimension so no on-device transposes are
needed: the host supplies x transposed (feature-major) and U/V pre-transposed
per block, all in fp16; y is stored fp16 and upcast + bias-added on the host.

Engine split: TensorE does both matmul stages back to back; VectorE rounds
stage-1 PSUM to f16 SBUF (CAST); ScalarE evicts stage-2 PSUM to SBUF so a
backed-up y-store DMA can never block the z-eviction path PE depends on.
A short burst of dummy matmuls on a zeroed tile runs during the initial DMA
wait so the PE's HAM clock gate is already released (2.4 GHz) when the first
real matmul issues.
"""

import numpy as np
from contextlib import ExitStack

NCORES = 8
TOK = 8192            # total tokens
D = 4096              # features
NB = 4                # num row/col blocks
BS = 1024             # block size
R = 256               # TT rank
TQ = TOK // 4         # tokens per core (2048)
TC = 512              # token chunk (moving dim for stage 1)
NCHUNK = TQ // TC     # 4 chunks

_CACHE = {}


def _build_nc():
    if "nc" in _CACHE:
        return _CACHE["nc"]

    import concourse.bacc as bacc
    import concourse.tile as tile
    import concourse.mybir as mybir

    dt = mybir.dt
    f16 = dt.float16

    nc = bacc.Bacc("TRN2", target_bir_lowering=False, debug=False)

    # x shard, host-tiled: [chunk, c, partition(v%128), vj*TC + t]
    xh_d = nc.dram_tensor("xh", [NCHUNK, NB, 128, 8 * TC], f16, kind="ExternalInput").ap()
    # V^T per (o_loc, c) block: [b, p(v%128), vj*256 + r]
    vt_d = nc.dram_tensor("vt", [8, 128, 2048], f16, kind="ExternalInput").ap()
    # U^T per (o_loc, c) block: [b, p(r%128), rj*1024 + m]
    ut_d = nc.dram_tensor("ut", [8, 128, 2048], f16, kind="ExternalInput").ap()
    # output shard: [2048 tokens, o_loc*1024 + m] (bias added host-side)
    y_d = nc.dram_tensor("y", [TQ, 2048], f16, kind="ExternalOutput").ap()

    with tile.TileContext(nc) as tcx, ExitStack() as ctx:
        wpool = ctx.enter_context(tcx.tile_pool(name="w", bufs=1))
        xpool = ctx.enter_context(tcx.tile_pool(name="xp", bufs=2))
        zpool = ctx.enter_context(tcx.tile_pool(name="zp", bufs=1))
        ypool = ctx.enter_context(tcx.tile_pool(name="yp", bufs=7))
        zps_pool = ctx.enter_context(tcx.tile_pool(name="zps", bufs=4, space="PSUM"))
        yps_pool = ctx.enter_context(tcx.tile_pool(name="yps", bufs=4, space="PSUM"))

        # HAM warmup: ~3.5us of tiny dummy matmuls on a zeroed tile run while
        # the first vt/x DMAs are in flight, so the PE clock gate is at 8/8 by
        # the time the first real matmul's inputs land. N=16 keeps each one
        # ~60ns so the handoff to the first real matmul wastes <100ns.
        warm = wpool.tile([128, 16], f16, tag="warm")
        nc.vector.memset(warm[:], 0.0)
        wps = zps_pool.tile([128, 512], dt.float32, tag="zps", name="warm_ps")
        for _ in range(56):
            nc.tensor.matmul(wps[0:16, 0:16], warm[:], warm[:], start=True, stop=True)

        # resident weights, DMA'd in exactly the order the compute needs them:
        # V^T block for c / the c-th x slice / the o1 V^T block, interleaved
        # through chunk-0 stage 1; U^T halves arrive during chunk-0 compute.
        vtt = [None] * 8
        utt = [None] * 8

        def load_vt(b):
            # quarter DMAs into a bufs=1 tile: region-granular deps let the
            # vj=0 matmul start before the whole block has landed
            t = wpool.tile([128, 2048], f16, tag=f"vt{b}", name=f"vt{b}")
            for q in range(4):
                nc.sync.dma_start(t[:, q * 512 : (q + 1) * 512], vt_d[b][:, q * 512 : (q + 1) * 512])
            vtt[b] = t

        def emit_stage1(tc_i):
            zsb = {}
            for c in range(NB):
                xc = xpool.tile([128, 8 * TC], f16, tag="xc")
                if tc_i == 0:
                    # chunk 0 is DMA-paced: interleave the o0 V^T block's
                    # quarters with the x quarters in exactly the order the
                    # stage-1 matmuls consume them (vt cols track x cols 2:1),
                    # with an extra-fine first pair for c==0 so the very first
                    # matmul's dep is ~160KB into the stream.
                    vtt[c] = wpool.tile([128, 2048], f16, tag=f"vt{c}", name=f"vt{c}")
                    if c == 0:
                        nc.sync.dma_start(vtt[c][:, 0:128], vt_d[c][:, 0:128])
                        nc.sync.dma_start(xc[:, 0:512], xh_d[tc_i, c, :, 0:512])
                        nc.sync.dma_start(vtt[c][:, 128:512], vt_d[c][:, 128:512])
                        nc.sync.dma_start(xc[:, 512:1024], xh_d[tc_i, c, :, 512:1024])
                    else:
                        nc.sync.dma_start(vtt[c][:, 0:512], vt_d[c][:, 0:512])
                        nc.sync.dma_start(xc[:, 0:1024], xh_d[tc_i, c, :, 0:1024])
                    for q in range(1, 4):
                        nc.sync.dma_start(vtt[c][:, q * 512 : (q + 1) * 512], vt_d[c][:, q * 512 : (q + 1) * 512])
                        cols = slice(q * 1024, (q + 1) * 1024)
                        nc.sync.dma_start(xc[:, cols], xh_d[tc_i, c, :, cols])
                    load_vt(4 + c)  # o1 block for this c
                else:
                    # quarter DMAs: region-granular WAR lets each quarter
                    # start as soon as the previous chunk's reads of that
                    # region finish, and stage-1 can start on a partial tile
                    for q in range(4):
                        cols = slice(q * 2 * TC, (q + 1) * 2 * TC)
                        nc.sync.dma_start(xc[:, cols], xh_d[tc_i, c, :, cols])
                for o in range(2):
                    b = o * 4 + c
                    for rj in range(2):
                        zps = zps_pool.tile([128, TC], dt.float32, tag="zps", name="zps")
                        for vj in range(8):
                            nc.tensor.matmul(
                                zps[:],
                                vtt[b][:, vj * 256 + rj * 128 : vj * 256 + rj * 128 + 128],
                                xc[:, vj * TC : (vj + 1) * TC],
                                start=(vj == 0),
                                stop=(vj == 7),
                            )
                        zt = zpool.tile([128, TC], f16, tag=f"z{b}_{rj}")
                        nc.vector.tensor_copy(zt[:], zps[:])
                        zsb[(b, rj)] = zt
            return zsb

        def emit_stage2(tc_i, zsb):
            for o in range(2):
                for mc in range(2):
                    for tt in range(TC // 128):
                        yps = yps_pool.tile([128, 512], dt.float32, tag="yps", name="yps")
                        k = 0
                        for c in range(NB):
                            b = o * 4 + c
                            for rj in range(2):
                                nc.tensor.matmul(
                                    yps[:],
                                    zsb[(b, rj)][:, tt * 128 : (tt + 1) * 128],
                                    utt[b][:, rj * 1024 + mc * 512 : rj * 1024 + mc * 512 + 512],
                                    start=(k == 0),
                                    stop=(k == 7),
                                )
                                k += 1
                        ysb = ypool.tile([128, 512], f16, tag="ysb")
                        t0 = tc_i * TC + tt * 128
                        col0 = o * 1024 + mc * 512
                        # y stores go out on the SWDGE path so they never queue
                        # ahead of the next chunk's x loads in the HWDGE FIFOs;
                        # the last chunk switches to HWDGE (no loads left to
                        # contend with) to skip the SWDGE drain at the tail
                        eng = nc.sync if tc_i == NCHUNK - 1 else nc.gpsimd
                        last = tc_i == NCHUNK - 1 and o == 1 and mc == 1 and tt == 3
                        if last:
                            # final group: evict + store in halves so the
                            # first half's store overlaps the second half's
                            # eviction (ScalarE+VectorE on one PSUM bank in
                            # parallel is unsupported, so both run on ScalarE)
                            nc.scalar.copy(ysb[:, 0:256], yps[:, 0:256])
                            eng.dma_start(y_d[t0 : t0 + 128, col0 : col0 + 256], ysb[:, 0:256])
                            nc.scalar.copy(ysb[:, 256:512], yps[:, 256:512])
                            eng.dma_start(y_d[t0 : t0 + 128, col0 + 256 : col0 + 512], ysb[:, 256:512])
                        else:
                            nc.scalar.copy(ysb[:], yps[:])
                            eng.dma_start(y_d[t0 : t0 + 128, col0 : col0 + 512], ysb[:])

        for tc_i in range(NCHUNK):
            zsb = emit_stage1(tc_i)
            if tc_i == 0:
                # U^T in quarter-block DMAs ordered by first use in stage 2:
                # mc=0 halves for every block first, then mc=1 halves.
                for b in range(8):
                    utt[b] = wpool.tile([128, 2048], f16, tag=f"ut{b}", name=f"ut{b}")
                for mc in range(2):
                    for b in range(8):
                        for rj in range(2):
                            cols = slice(rj * 1024 + mc * 512, rj * 1024 + mc * 512 + 512)
                            nc.sync.dma_start(utt[b][:, cols], ut_d[b][:, cols])
            emit_stage2(tc_i, zsb)

    nc.compile()
    _CACHE["nc"] = nc
    return nc


def _prep_in_maps(x, U, V, bias):
    x = np.ascontiguousarray(x, dtype=np.float32).reshape(TOK, D).astype(np.float16)
    U = np.asarray(U, dtype=np.float32).astype(np.float16)
    V = np.asarray(V, dtype=np.float32).astype(np.float16)

    # xh[tc, c, p, vj, tt] = x[tq*2048 + tc*TC + tt, c*1024 + vj*128 + p]
    xhs = []
    for tq in range(4):
        shard = x[tq * TQ : (tq + 1) * TQ]  # [2048, 4096]
        xh = shard.reshape(NCHUNK, TC, NB, 8, 128).transpose(0, 2, 4, 3, 1)
        xhs.append(np.ascontiguousarray(xh).reshape(NCHUNK, NB, 128, 8 * TC))

    vts, uts = [], []
    for og in range(2):
        Vg = V[og * 2 : og * 2 + 2]  # [2, 4, 256, 1024]
        vt = Vg.reshape(2, NB, 256, 8, 128).transpose(0, 1, 4, 3, 2)
        vts.append(np.ascontiguousarray(vt).reshape(8, 128, 2048))
        Ug = U[og * 2 : og * 2 + 2]  # [2, 4, 1024, 256]
        ut = Ug.reshape(2, NB, 1024, 2, 128).transpose(0, 1, 4, 3, 2)
        uts.append(np.ascontiguousarray(ut).reshape(8, 128, 2048))

    in_maps = []
    for g in range(NCORES):
        og, tq = g // 4, g % 4
        in_maps.append({"xh": xhs[tq], "vt": vts[og], "ut": uts[og]})
    return in_maps


def _assemble(results, bias):
    y = np.empty((TOK, D), dtype=np.float32)
    for g in range(NCORES):
        og, tq = g // 4, g % 4
        y[tq * TQ : (tq + 1) * TQ, og * 2048 : (og + 1) * 2048] = results[g]["y"]
    y = y.reshape(TOK // 2048, 2048, NB, BS)
    y += np.asarray(bias, dtype=np.float32)[None, None, :, :]
    return y.reshape(4, 2048, D)


def run_with_options(inputs, trace=False, **kw):
    from concourse.bass_utils import run_bass_kernel_spmd

    nc = _build_nc()
    in_maps = _prep_in_maps(**inputs)
    res = run_bass_kernel_spmd(nc, in_maps, core_ids=list(range(NCORES)), trace=trace, **kw)
    return _assemble(res.results, inputs["bias"]), res


def kernel(x, U, V, bias):
    out, _ = run_with_options({"x": x, "U": U, "V": V, "bias": bias})
    return out
